# revision 16
# baseline (speedup 1.0000x reference)
"""Trainium2 Bass kernel for nn_KResampleRenderer_78967268704313.

Math
----
The reference resamples a Hermitian half-plane Fourier image
(C=8, 2048, 1025) onto a (1025, 513) output k-grid with a 6x6 quintic
interpolation stencil, then multiplies by the interpolant's Fourier
transform and ifftshifts. The resample coordinates
  kx = linspace(0, 512, 513),  ky = linspace(-512, 512, 1025)
are exactly integer-valued (kmax = 2048/2 * 0.05/0.1 = 512.0 exactly in
both f64 and f32), and the quintic kernel is an interpolant
(quintic(0)=1, quintic(n)=0 for integer n!=0), so the whole stencil
collapses to a gather of input rows/cols. Folding in fftshift (axis -2
of the input), the Hermitian indexing (all requested kx >= 0 -> no
conjugation), and the final ifftshift (axis -2, N=1025 odd), the
reference is exactly:

    out[ch, i, c] = kimage[ch, src(i), c] * fy[(i+512) % 1025] * fx[c]

    src(i) = i            for i in [0, 512]
           = i + 1023     for i in [513, 1024]
    fx[c] = quintic_uval(ux[c] / 2pi),  ux = linspace(0, pi, 513) * 0.5
    fy[r] = quintic_uval(uy[r] / 2pi),  uy = linspace(-pi, pi, 1025)

(verified numerically against the jax reference: Frobenius rel err
3.3e-6, pure f32 rounding noise).

Sharding
--------
Embarrassingly parallel over channels: 8 channels onto 8 cores, one
channel each. The host packs, per channel, the 1025 needed rows x 513
needed cols of real/imag (the row gather is two contiguous slices) into
one (1025, 1026) array with [real | imag] packed per row, plus two
small weight vectors. The host splits the returned (1025, 1026) plane
pair back into complex64.

Device kernel (per core)
------------------------
Main 1024 rows live as row = 8p + rw (partition p, 0<=rw<8), so every
DMA moves 4104B-contiguous per-partition chunks. The weight tile
W[p, rw*513+c] = fy[8p+rw] * fx[c] is built on-chip once (8
tensor_scalar ops from two tiny consts), then each of 8 row-groups is
load -> 2x tensor_mul (real/imag columns) -> store. Loads ride the SP
HWDGE ring, stores + consts the ACT ring, compute on DVE; ~28us
predicted by the timeline cost model, within ~15% of the 8.4MB/core
HBM roofline.

A DMA-completion wait is only exact when the awaited count covers every
increment ever issued to that semaphore so far - a shared cumulative
counter can hit an intermediate threshold while a straggler SDMA engine
still hasn't landed this DMA's partitions (observed as corrupted
trailing partitions). Every DMA therefore gets a dedicated semaphore.

Raw Bass rather than TileContext: the Tile kernel-tail drain emits more
sync-waits than this walrus build encodes ("Too many sync wait
commands").
"""

from contextlib import ExitStack

import numpy as np

import concourse.bass as bass
import concourse.mybir as mybir
from concourse.bass_utils import run_bass_kernel_spmd

N_CH = 8
SO = 1025  # output rows
HC = 513  # output cols (kx >= 0 half plane)
RW = 8  # rows per partition for the main 1024 rows
G = 8  # pipeline groups (R = RW // G rows-per-partition each)
IN_RES = 0.05
OUT_RES = 0.1


def _quintic_uval(u):
    """Fourier transform of the quintic interpolant, float64."""
    u = np.abs(np.asarray(u, dtype=np.float64))
    piu = np.pi * u
    small = np.abs(piu) < 1e-6
    safe = np.where(small, 1.0, piu)
    s = np.where(small, 1.0 - piu * piu / 6.0, np.sin(safe) / safe)
    c = np.cos(piu)
    piusq = piu * piu
    ssq = s * s
    return s * ssq * ssq * (s * (55.0 - 19.0 * piusq) + 2.0 * c * (piusq - 27.0))


def _weights():
    """fxb (128, 513) fx broadcast; fys (128, 9): [:, :8] = fy_shifted in
    row = 8p+rw order, [0, 8] = fy_shifted[1024] for the ragged last row."""
    ux = np.linspace(0.0, np.pi, HC) * (IN_RES / OUT_RES)
    uy = np.linspace(-np.pi, np.pi, SO)
    fx = _quintic_uval(ux / (2.0 * np.pi)).astype(np.float32)
    fy = _quintic_uval(uy / (2.0 * np.pi)).astype(np.float32)
    fy_sh = fy[(np.arange(SO) + SO // 2) % SO]  # ifftshift of the weight rows
    fys = np.zeros((128, RW + 1), dtype=np.float32)
    fys[:, :RW] = fy_sh[:1024].reshape(128, RW)
    fys[0, RW] = fy_sh[1024]
    fxb = np.ascontiguousarray(np.broadcast_to(fx, (128, HC)))
    return fxb, fys


def _build_nc(g_groups=G):
    assert RW % g_groups == 0
    R = RW // g_groups
    nc = bass.Bass()
    f32 = mybir.dt.float32
    z2 = nc.dram_tensor("z2", [SO, 2 * HC], f32, kind="ExternalInput")
    fys = nc.dram_tensor("fys", [128, RW + 1], f32, kind="ExternalInput")
    fxb = nc.dram_tensor("fxb", [128, HC], f32, kind="ExternalInput")
    o2 = nc.dram_tensor("o2", [SO, 2 * HC], f32, kind="ExternalOutput")
    mult = mybir.AluOpType.mult
    CW = 2 * HC  # packed row width (1026)
    SLOT = R * CW  # elements per partition per group slot

    with ExitStack() as ctx:
        fys_t = ctx.enter_context(nc.sbuf_tensor("fys_t", [128, RW + 1], f32))
        fx_t = ctx.enter_context(nc.sbuf_tensor("fx_t", [128, HC], f32))
        w_t = ctx.enter_context(nc.sbuf_tensor("w_t", [128, RW * HC], f32))
        zt = ctx.enter_context(nc.sbuf_tensor("zt", [128, g_groups * SLOT], f32))
        ot = ctx.enter_context(nc.sbuf_tensor("ot", [128, g_groups * SLOT], f32))
        zr9 = ctx.enter_context(nc.sbuf_tensor("zr9", [1, CW], f32))
        or9 = ctx.enter_context(nc.sbuf_tensor("or9", [1, CW], f32))
        const_sem = ctx.enter_context(nc.semaphore("const_sem"))
        v_sem = ctx.enter_context(nc.semaphore("v_sem"))
        zs = [ctx.enter_context(nc.semaphore(f"zs{g}")) for g in range(g_groups + 1)]
        os_ = [ctx.enter_context(nc.semaphore(f"os{g}")) for g in range(g_groups + 1)]
        block = ctx.enter_context(nc.Block())

        # main-row views: row = 8p + rw
        z3 = z2[:1024, :].rearrange("(p rw) c -> p rw c", p=128)
        o3 = o2[:1024, :].rearrange("(p rw) c -> p rw c", p=128)

        @block.sync
        def _(sync):
            for g in range(g_groups):
                sync.dma_start(
                    out=zt[:, g * SLOT : (g + 1) * SLOT],
                    in_=z3[:, g * R : (g + 1) * R, :],
                ).then_inc(zs[g], 16)
            sync.dma_start(out=zr9[:, :], in_=z2[1024:1025, :]).then_inc(
                zs[g_groups], 16
            )

        @block.vector
        def _(vector):
            vector.wait_ge(const_sem, 32)
            # build W[p, rw*513+c] = fys[p, rw] * fx[c]
            for rw in range(RW):
                vector.tensor_scalar_mul(
                    w_t[:, rw * HC : (rw + 1) * HC],
                    fx_t[:, :],
                    fys_t[:, rw : rw + 1],
                )
            for g in range(g_groups):
                vector.wait_ge(zs[g], 16)
                z3s = zt[:, g * SLOT : (g + 1) * SLOT].rearrange(
                    "p (rw c) -> p rw c", c=CW
                )
                o3s = ot[:, g * SLOT : (g + 1) * SLOT].rearrange(
                    "p (rw c) -> p rw c", c=CW
                )
                w3s = w_t[:, g * R * HC : (g + 1) * R * HC].rearrange(
                    "p (rw c) -> p rw c", c=HC
                )
                # real plane at column offset 0, imag at +HC within each row
                for off in (0, HC):
                    vector.tensor_mul(
                        o3s[:, :, off : off + HC],
                        z3s[:, :, off : off + HC],
                        w3s[:, :, :],
                    ).then_inc(v_sem, 1)
            # ragged row 1024
            vector.wait_ge(zs[g_groups], 16)
            for off in (0, HC):
                vector.scalar_tensor_tensor(
                    out=or9[0:1, off : off + HC],
                    in0=zr9[0:1, off : off + HC],
                    scalar=fys_t[0:1, RW : RW + 1],
                    in1=fx_t[0:1, :],
                    op0=mult,
                    op1=mult,
                ).then_inc(v_sem, 1)

        @block.scalar
        def _(scalar):
            # consts ride the store ring, idle at kernel start - keeps the
            # load ring on data from t=0
            scalar.dma_start(out=fys_t[:, :], in_=fys[:, :]).then_inc(const_sem, 16)
            scalar.dma_start(out=fx_t[:, :], in_=fxb[:, :]).then_inc(const_sem, 16)
            for g in range(g_groups):
                scalar.wait_ge(v_sem, 2 * (g + 1))
                scalar.dma_start(
                    out=o3[:, g * R : (g + 1) * R, :],
                    in_=ot[:, g * SLOT : (g + 1) * SLOT],
                ).then_inc(os_[g], 16)
            scalar.wait_ge(v_sem, 2 * g_groups + 2)
            scalar.dma_start(out=o2[1024:1025, :], in_=or9[:, :]).then_inc(
                os_[g_groups], 16
            )
            for g in range(g_groups + 1):
                scalar.wait_ge(os_[g], 16)

    return nc


_NC_CACHE = None


def _get_nc():
    global _NC_CACHE
    if _NC_CACHE is None:
        _NC_CACHE = _build_nc()
    return _NC_CACHE


def _in_maps(kr, ki):
    fxb, fys = _weights()
    in_maps = []
    for ch in range(N_CH):
        # src rows [0..512] ++ [1536..2047], cols [0..512]
        zr_sel = np.concatenate((kr[ch, :HC, :HC], kr[ch, 1536:, :HC]), axis=0)
        zi_sel = np.concatenate((ki[ch, :HC, :HC], ki[ch, 1536:, :HC]), axis=0)
        z2 = np.concatenate((zr_sel, zi_sel), axis=1)  # (1025, 1026)
        in_maps.append({"z2": np.ascontiguousarray(z2), "fys": fys, "fxb": fxb})
    return in_maps


def _run(kimage_real, kimage_imag, trace=False):
    kr = np.ascontiguousarray(np.asarray(kimage_real, dtype=np.float32))
    ki = np.ascontiguousarray(np.asarray(kimage_imag, dtype=np.float32))
    assert kr.shape == (N_CH, 2048, 1025), kr.shape

    res = run_bass_kernel_spmd(
        _get_nc(), _in_maps(kr, ki), core_ids=list(range(N_CH)), trace=trace
    )

    out = np.empty((N_CH, SO, HC), dtype=np.complex64)
    for ch in range(N_CH):
        o2 = res.results[ch]["o2"]
        out.real[ch] = o2[:, :HC]
        out.imag[ch] = o2[:, HC:]
    return out, res


def kernel(kimage_real, kimage_imag):
    out, _ = _run(kimage_real, kimage_imag)
    return out
